# revision 7
# baseline (speedup 1.0000x reference)
"""GCN layer (gather + scale + segment-sum + 128x128 matmul) on 8 TRN2 NeuronCores.

Sharding: nodes (and their incident edges, partitioned by dst) are sharded
across the 8 cores; the 128x128 weight is replicated. Per core (~100K edges):

  host (integer/permutation preprocessing only):
    - select edges with dst in the core's 6250-row slice, sort by dst block
      so each 128-edge tile maps to one 128-dst block
    - pad each block's edge group to a multiple of 128 with
      (idx=0, w=0, dst_local=-1); per-block tile counts are maxed across
      cores so all 8 cores run one SPMD program
    - gather indices address PAIRS of feat rows (idx = src>>1, 1KB elems) so
      indices fit int16 without a low/high base split; a 0/1 parity mask
      per edge selects the row within the pair
    - ship per-edge out-degree counts / per-node in-degree counts (integer
      index bookkeeping); all float math happens on device

  device:
    - s_e = edge_w * rsqrt(outdeg[src_e]); mlow/mhigh = s_e * (1-par)/par (DVE)
    - batched dma_gather of feat row-pairs by src>>1 (1KB rows)  (SWDGE Q7)
    - per tile: two fp16 casts of the pair halves scaled by mlow/mhigh (ACT)
    - selector tile sel[e, d] = (iota[d]==dst_local[e])  (one DVE op)
    - aggT[f, d] += mlow_tile^T @ sel + mhigh_tile^T @ sel  in PSUM  (PE)
    - rst = (aggT^T @ W) * rsqrt(max(indeg,1)) + b  (PE + ACT + DVE)
"""

import os
import numpy as np

N_NODES = 50000
N_EDGES = 800000
F = 128
C = 8
NPC = N_NODES // C          # 6250 nodes per core
NB = (NPC + 127) // 128     # 49 dst blocks per core (48 full + 106)
CHUNK_T = 8                 # tiles per dma_gather (1024-idx ucode packet limit)


def _host_prep(feat, W, b, edge_w, edge_src, edge_dst):
    src = np.ascontiguousarray(np.asarray(edge_src)).astype(np.int64)
    dst = np.ascontiguousarray(np.asarray(edge_dst)).astype(np.int64)
    w = np.ascontiguousarray(np.asarray(edge_w)).astype(np.float32)

    outcnt = np.bincount(src, minlength=N_NODES)

    per_core = []
    core_of = dst // NPC
    for c in range(C):
        m = core_of == c
        s_c = src[m]
        d_c = dst[m] - c * NPC
        w_c = w[m]
        blk = d_c >> 7
        perm = np.argsort(blk, kind="stable")
        per_core.append((s_c[perm], d_c[perm], w_c[perm], blk[perm]))

    cnts = np.zeros((C, NB), np.int64)
    for c in range(C):
        _, _, _, blk = per_core[c]
        cnts[c] = np.bincount(blk, minlength=NB)
    T_b = np.maximum(1, (cnts.max(axis=0) + 127) // 128)  # [NB]
    off = np.zeros(NB, np.int64)
    cur = 0
    for bk in range(NB):
        off[bk] = cur
        cur += T_b[bk]
    T_total = int(cur)

    in_maps = []
    for c in range(C):
        s_c, d_c, w_c, blk = per_core[c]
        gidx = np.zeros(T_total * 128, np.int64)
        wv = np.zeros(T_total * 128, np.float32)
        par = np.zeros(T_total * 128, np.int64)
        dstl = np.full(T_total * 128, -1, np.int64)
        cnt = np.ones(T_total * 128, np.int64)
        grp_cnt = np.bincount(blk, minlength=NB)
        grp_start = np.concatenate([[0], np.cumsum(grp_cnt)])
        for bk in range(NB):
            e0, e1 = grp_start[bk], grp_start[bk + 1]
            k = e1 - e0
            s0 = off[bk] * 128
            gidx[s0:s0 + k] = s_c[e0:e1] >> 1
            par[s0:s0 + k] = s_c[e0:e1] & 1
            wv[s0:s0 + k] = w_c[e0:e1]
            dstl[s0:s0 + k] = d_c[e0:e1] - bk * 128
            cnt[s0:s0 + k] = outcnt[s_c[e0:e1]]
        icnt = np.bincount(d_c, minlength=NPC)
        icnt_pad = np.ones(NB * 128, np.int64)
        icnt_pad[:NPC] = icnt

        idx_wrapped = np.tile(gidx.reshape(-1, 16).T.astype(np.int16), (8, 1))

        in_maps.append({
            "feat": np.ascontiguousarray(np.asarray(feat, np.float32)),
            "Wm": np.ascontiguousarray(np.asarray(W, np.float32)),
            "bv": np.ascontiguousarray(np.asarray(b, np.float32).reshape(1, F)),
            "wv": np.ascontiguousarray(wv.reshape(T_total, 128).T),
            "parv": np.ascontiguousarray(par.reshape(T_total, 128).T.astype(np.float32)),
            "dstl": np.ascontiguousarray(dstl.reshape(T_total, 128).T.astype(np.float32)),
            "ocnt": np.ascontiguousarray(cnt.reshape(T_total, 128).T.astype(np.int16)),
            "icnt": np.ascontiguousarray(icnt_pad.reshape(NB, 128).T.astype(np.int16)),
            "gidx": np.ascontiguousarray(idx_wrapped),
        })
    return T_b, off, in_maps


_BUILD_CACHE = {}


def _build_program(T_b, off):
    import concourse.bacc as bacc
    import concourse.mybir as mybir
    import concourse.tile as tile
    from concourse._compat import get_trn_type

    dt = mybir.dt
    AF = mybir.ActivationFunctionType
    ALU = mybir.AluOpType

    T_total = int(T_b.sum())

    nc = bacc.Bacc(get_trn_type() or "TRN2", target_bir_lowering=False, debug=False)

    feat_d = nc.dram_tensor("feat", [N_NODES, F], dt.float32, kind="ExternalInput")
    W_d = nc.dram_tensor("Wm", [F, F], dt.float32, kind="ExternalInput")
    b_d = nc.dram_tensor("bv", [1, F], dt.float32, kind="ExternalInput")
    wv_d = nc.dram_tensor("wv", [128, T_total], dt.float32, kind="ExternalInput")
    par_d = nc.dram_tensor("parv", [128, T_total], dt.float32, kind="ExternalInput")
    dstl_d = nc.dram_tensor("dstl", [128, T_total], dt.float32, kind="ExternalInput")
    ocnt_d = nc.dram_tensor("ocnt", [128, T_total], dt.int16, kind="ExternalInput")
    icnt_d = nc.dram_tensor("icnt", [128, NB], dt.int16, kind="ExternalInput")
    gidx_d = nc.dram_tensor("gidx", [128, T_total * 8], dt.int16, kind="ExternalInput")
    out_d = nc.dram_tensor("out", [NPC, F], dt.float32, kind="ExternalOutput")

    feat_pairs = feat_d[:, :].rearrange("(n k) f -> n (k f)", k=2)  # [25000, 256]

    # tile -> block mapping
    tile_blk = np.repeat(np.arange(NB), T_b)
    tile_in_blk = np.concatenate([np.arange(T_b[bk]) for bk in range(NB)])

    with tile.TileContext(nc) as tc:
        with (
            tc.tile_pool(name="const", bufs=1) as cpool,
            tc.tile_pool(name="gbuf", bufs=8) as gpool,
            tc.tile_pool(name="mbuf", bufs=24) as mpool,
            tc.tile_pool(name="sel", bufs=12) as spool,
            tc.tile_pool(name="rst", bufs=3) as rpool,
            tc.tile_pool(name="pacc", bufs=3, space="PSUM") as papool,
            tc.tile_pool(name="prst", bufs=2, space="PSUM") as prpool,
        ):
            # ---- constant / setup loads ----
            w_sb = cpool.tile([128, T_total], dt.float32)
            par_sb = cpool.tile([128, T_total], dt.float32)
            dstl_sb = cpool.tile([128, T_total], dt.float32)
            ocnt_sb = cpool.tile([128, T_total], dt.int16)
            icnt_sb = cpool.tile([128, NB], dt.int16)
            gidx_sb = cpool.tile([128, T_total * 8], dt.int16)
            W_sb = cpool.tile([128, F], dt.float32)
            W_h = cpool.tile([128, F], dt.float16)
            b_sb = cpool.tile([1, F], dt.float32)
            ones1 = cpool.tile([1, F], dt.float32)
            b_bcast = cpool.tile([128, F], dt.float32)
            iota_h = cpool.tile([128, 128], dt.float16)
            mlow = cpool.tile([128, T_total], dt.float32)
            mhigh = cpool.tile([128, T_total], dt.float32)
            rs_in = cpool.tile([128, NB], dt.float32)
            aggTh = cpool.tile([128, NB * 128], dt.float16)
            tmp_f = cpool.tile([128, T_total], dt.float32)
            tmp_g = cpool.tile([128, T_total], dt.float32)
            tmp_i = cpool.tile([128, NB], dt.float32)
            tmp_j = cpool.tile([128, NB], dt.float32)

            nc.sync.dma_start(out=w_sb[:], in_=wv_d[:])
            nc.sync.dma_start(out=par_sb[:], in_=par_d[:])
            nc.sync.dma_start(out=dstl_sb[:], in_=dstl_d[:])
            nc.sync.dma_start(out=ocnt_sb[:], in_=ocnt_d[:])
            nc.sync.dma_start(out=icnt_sb[:], in_=icnt_d[:])
            nc.sync.dma_start(out=gidx_sb[:], in_=gidx_d[:])
            nc.sync.dma_start(out=W_sb[:], in_=W_d[:])
            nc.sync.dma_start(out=b_sb[:], in_=b_d[:])

            nc.scalar.activation(out=W_h[:], in_=W_sb[:], func=AF.Copy)

            # broadcast b across partitions via a K=1 outer-product matmul
            nc.vector.memset(ones1[:], 1.0)
            pb = prpool.tile([128, F], dt.float32, space="PSUM")
            nc.tensor.matmul(pb[:], ones1[:], b_sb[:], start=True, stop=True)
            nc.vector.tensor_copy(out=b_bcast[:], in_=pb[:])

            nc.gpsimd.iota(
                iota_h[:], pattern=[[1, 128]], base=0, channel_multiplier=0,
                allow_small_or_imprecise_dtypes=True,
            )

            # s_e = w_e * rsqrt(outdeg_e); split by pair parity into mlow/mhigh
            nc.vector.tensor_copy(out=tmp_f[:], in_=ocnt_sb[:])
            nc.vector.reciprocal(out=tmp_g[:], in_=tmp_f[:])
            nc.scalar.activation(out=tmp_f[:], in_=tmp_g[:], func=AF.Sqrt)
            nc.vector.tensor_tensor(out=tmp_g[:], in0=w_sb[:], in1=tmp_f[:], op=ALU.mult)
            nc.vector.tensor_tensor(out=mhigh[:], in0=tmp_g[:], in1=par_sb[:], op=ALU.mult)
            nc.vector.tensor_tensor(out=mlow[:], in0=tmp_g[:], in1=mhigh[:], op=ALU.subtract)

            # rs_in = rsqrt(max(indeg, 1))
            nc.vector.tensor_copy(out=tmp_i[:], in_=icnt_sb[:])
            nc.vector.tensor_scalar_max(tmp_j[:], tmp_i[:], 1.0)
            nc.vector.reciprocal(out=tmp_i[:], in_=tmp_j[:])
            nc.scalar.activation(out=rs_in[:], in_=tmp_i[:], func=AF.Sqrt)

            # ---- main aggregation ----
            pa = None
            for c0 in range(0, T_total, CHUNK_T):
                ct = min(CHUNK_T, T_total - c0)
                gbuf = gpool.tile([128, CHUNK_T * 256], dt.float32, tag="gbuf")
                nc.gpsimd.dma_gather(
                    gbuf[:, :ct * 256].rearrange("p (t e) -> p t e", e=256),
                    feat_pairs,
                    gidx_sb[:, c0 * 8:(c0 + ct) * 8],
                    ct * 128,
                    ct * 128,
                    256,
                )
                for t in range(ct):
                    gt = c0 + t
                    bk = int(tile_blk[gt])
                    ti = int(tile_in_blk[gt])
                    last = ti == T_b[bk] - 1
                    ml = mpool.tile([128, 128], dt.float16, tag="ml")
                    mh = mpool.tile([128, 128], dt.float16, tag="mh")
                    nc.scalar.activation(
                        out=ml[:], in_=gbuf[:, t * 256:t * 256 + 128],
                        func=AF.Copy, scale=mlow[:, gt:gt + 1],
                    )
                    nc.scalar.activation(
                        out=mh[:], in_=gbuf[:, t * 256 + 128:t * 256 + 256],
                        func=AF.Copy, scale=mhigh[:, gt:gt + 1],
                    )
                    sel = spool.tile([128, 128], dt.float16, tag="sel")
                    nc.vector.tensor_scalar(
                        sel[:], iota_h[:], dstl_sb[:, gt:gt + 1], None,
                        op0=ALU.is_equal,
                    )
                    if ti == 0:
                        pa = papool.tile([128, 128], dt.float32, space="PSUM", tag="pa")
                    nc.tensor.matmul(pa[:], ml[:], sel[:], start=(ti == 0), stop=False)
                    nc.tensor.matmul(pa[:], mh[:], sel[:], start=False, stop=last)
                    if last:
                        bs = slice(bk * 128, (bk + 1) * 128)
                        nc.scalar.activation(out=aggTh[:, bs], in_=pa[:], func=AF.Copy)

            # ---- finalize: rst = (aggT^T @ W) * rs_in + b ----
            for bk in range(NB):
                bs = slice(bk * 128, (bk + 1) * 128)
                pr = prpool.tile([128, F], dt.float32, space="PSUM", tag="pr")
                nc.tensor.matmul(pr[:], aggTh[:, bs], W_h[:], start=True, stop=True)
                rt = rpool.tile([128, F], dt.float32, tag="rt")
                nc.scalar.activation(
                    out=rt[:], in_=pr[:], func=AF.Copy, scale=rs_in[:, bk:bk + 1],
                )
                nc.vector.tensor_tensor(out=rt[:], in0=rt[:], in1=b_bcast[:], op=ALU.add)
                n0 = bk * 128
                n1 = min(n0 + 128, NPC)
                nc.sync.dma_start(out=out_d[n0:n1, :], in_=rt[:n1 - n0, :])

    nc.compile()
    return nc


def kernel(feat, W, b, edge_w, edge_src, edge_dst):
    from concourse.bass_utils import run_bass_kernel_spmd

    T_b, off, in_maps = _host_prep(feat, W, b, edge_w, edge_src, edge_dst)

    key = tuple(T_b)
    if key not in _BUILD_CACHE:
        _BUILD_CACHE[key] = _build_program(T_b, off)
    nc = _BUILD_CACHE[key]

    trace = bool(int(os.environ.get("GCN_TRACE", "0")))
    res = run_bass_kernel_spmd(
        nc, in_maps, core_ids=list(range(C)),
        trace=trace,
        trace_cores=list(range(C)) if trace else None,
    )
    kernel.last_results = res
    out = np.concatenate([r["out"] for r in res.results], axis=0)
    return out.astype(np.float32)


# revision 8
# speedup vs baseline: 1.1666x; 1.1666x over previous
"""GCN layer (gather + scale + segment-sum + 128x128 matmul) on 8 TRN2 NeuronCores.

Sharding: nodes (and their incident edges, partitioned by dst) are sharded
across the 8 cores; the 128x128 weight is replicated. Per core (~100K edges):

  host (integer/permutation/structure preprocessing only):
    - select edges with dst in the core's 6250-row slice, sort by dst block
      so each 128-edge tile maps to one 128-dst block
    - pad each block's edge group to a multiple of 128 with
      (idx=0, w=0, empty selector row); per-block tile counts are maxed
      across cores so all 8 cores run one SPMD program
    - gather indices address PAIRS of feat rows (idx = src>>1, 1KB elems) so
      indices fit int16; a 0/1 parity mask per edge selects the row in the
      pair
    - the 0/1 one-hot selector tiles (edge -> dst_local routing, pure graph
      structure) are shipped as an fp16 input and streamed by regular DMA
    - per-edge out-degree counts / per-node in-degree counts are shipped as
      int16 (index bookkeeping); all float math happens on device

  device:
    - s_e = edge_w * rsqrt(outdeg[src_e]); mlow/mhigh = s_e * (1-par)/par (DVE)
    - batched dma_gather of feat row-pairs by src>>1 (1KB rows)  (SWDGE Q7)
    - per tile: fp16 cast of pair halves scaled by mlow (ACT) / mhigh (DVE)
    - aggT[f, d] += mlow_tile^T @ sel + mhigh_tile^T @ sel  in PSUM  (PE)
    - per block: rst = (aggT^T @ W) * rsqrt(max(indeg,1)) + b, finalized
      inline so it overlaps the gather stream  (PE + ACT + DVE)
"""

import os
import numpy as np

N_NODES = 50000
N_EDGES = 800000
F = 128
C = 8
NPC = N_NODES // C          # 6250 nodes per core
NB = (NPC + 127) // 128     # 49 dst blocks per core (48 full + 106)
CHUNK_T = 8                 # tiles per dma_gather (1024-idx ucode packet limit)


def _host_prep(feat, W, b, edge_w, edge_src, edge_dst):
    src = np.ascontiguousarray(np.asarray(edge_src)).astype(np.int64)
    dst = np.ascontiguousarray(np.asarray(edge_dst)).astype(np.int64)
    w = np.ascontiguousarray(np.asarray(edge_w)).astype(np.float32)

    outcnt = np.bincount(src, minlength=N_NODES)

    per_core = []
    core_of = dst // NPC
    for c in range(C):
        m = core_of == c
        s_c = src[m]
        d_c = dst[m] - c * NPC
        w_c = w[m]
        blk = d_c >> 7
        perm = np.argsort(blk, kind="stable")
        per_core.append((s_c[perm], d_c[perm], w_c[perm], blk[perm]))

    cnts = np.zeros((C, NB), np.int64)
    for c in range(C):
        _, _, _, blk = per_core[c]
        cnts[c] = np.bincount(blk, minlength=NB)
    T_b = np.maximum(1, (cnts.max(axis=0) + 127) // 128)  # [NB]
    off = np.zeros(NB, np.int64)
    cur = 0
    for bk in range(NB):
        off[bk] = cur
        cur += T_b[bk]
    T_total = int(cur)

    in_maps = []
    for c in range(C):
        s_c, d_c, w_c, blk = per_core[c]
        gidx = np.zeros(T_total * 128, np.int64)
        wv = np.zeros(T_total * 128, np.float32)
        par = np.zeros(T_total * 128, np.int64)
        dstl = np.full(T_total * 128, -1, np.int64)
        cnt = np.ones(T_total * 128, np.int64)
        grp_cnt = np.bincount(blk, minlength=NB)
        grp_start = np.concatenate([[0], np.cumsum(grp_cnt)])
        for bk in range(NB):
            e0, e1 = grp_start[bk], grp_start[bk + 1]
            k = e1 - e0
            s0 = off[bk] * 128
            gidx[s0:s0 + k] = s_c[e0:e1] >> 1
            par[s0:s0 + k] = s_c[e0:e1] & 1
            wv[s0:s0 + k] = w_c[e0:e1]
            dstl[s0:s0 + k] = d_c[e0:e1] - bk * 128
            cnt[s0:s0 + k] = outcnt[s_c[e0:e1]]
        icnt = np.bincount(d_c, minlength=NPC)
        icnt_pad = np.ones(NB * 128, np.int64)
        icnt_pad[:NPC] = icnt

        idx_wrapped = np.tile(gidx.reshape(-1, 16).T.astype(np.int16), (8, 1))

        # one-hot selector tiles: sel[128e, T*128] fp16, laid out tile-major.
        # sel_host[e, t*128 + d] = 1.0 iff dstl[t*128+e] == d
        dstl_t = dstl.reshape(T_total, 128)          # [T, e]
        sel_host = np.zeros((128, T_total * 128), np.float16)
        tt, ee = np.nonzero(dstl_t >= 0)
        sel_host[ee, tt * 128 + dstl_t[tt, ee]] = np.float16(1.0)

        in_maps.append({
            "feat": np.ascontiguousarray(np.asarray(feat, np.float32)),
            "Wm": np.ascontiguousarray(np.asarray(W, np.float32)),
            "bv": np.ascontiguousarray(np.asarray(b, np.float32).reshape(1, F)),
            "wv": np.ascontiguousarray(wv.reshape(T_total, 128).T),
            "parv": np.ascontiguousarray(par.reshape(T_total, 128).T.astype(np.float32)),
            "ocnt": np.ascontiguousarray(cnt.reshape(T_total, 128).T.astype(np.int16)),
            "icnt": np.ascontiguousarray(icnt_pad.reshape(NB, 128).T.astype(np.int16)),
            "gidx": np.ascontiguousarray(idx_wrapped),
            "selh": np.ascontiguousarray(sel_host),
        })
    return T_b, off, in_maps


_BUILD_CACHE = {}


def _build_program(T_b, off):
    import concourse.bacc as bacc
    import concourse.mybir as mybir
    import concourse.tile as tile
    from concourse._compat import get_trn_type

    dt = mybir.dt
    AF = mybir.ActivationFunctionType
    ALU = mybir.AluOpType

    T_total = int(T_b.sum())

    nc = bacc.Bacc(get_trn_type() or "TRN2", target_bir_lowering=False, debug=False)

    feat_d = nc.dram_tensor("feat", [N_NODES, F], dt.float32, kind="ExternalInput")
    W_d = nc.dram_tensor("Wm", [F, F], dt.float32, kind="ExternalInput")
    b_d = nc.dram_tensor("bv", [1, F], dt.float32, kind="ExternalInput")
    wv_d = nc.dram_tensor("wv", [128, T_total], dt.float32, kind="ExternalInput")
    par_d = nc.dram_tensor("parv", [128, T_total], dt.float32, kind="ExternalInput")
    ocnt_d = nc.dram_tensor("ocnt", [128, T_total], dt.int16, kind="ExternalInput")
    icnt_d = nc.dram_tensor("icnt", [128, NB], dt.int16, kind="ExternalInput")
    gidx_d = nc.dram_tensor("gidx", [128, T_total * 8], dt.int16, kind="ExternalInput")
    sel_d = nc.dram_tensor("selh", [128, T_total * 128], dt.float16, kind="ExternalInput")
    out_d = nc.dram_tensor("out", [NPC, F], dt.float32, kind="ExternalOutput")

    feat_pairs = feat_d[:, :].rearrange("(n k) f -> n (k f)", k=2)  # [25000, 256]

    tile_blk = np.repeat(np.arange(NB), T_b)
    tile_in_blk = np.concatenate([np.arange(T_b[bk]) for bk in range(NB)])

    with tile.TileContext(nc) as tc:
        with (
            tc.tile_pool(name="const", bufs=1) as cpool,
            tc.tile_pool(name="gbuf", bufs=10) as gpool,
            tc.tile_pool(name="selbuf", bufs=8) as selpool,
            tc.tile_pool(name="mbuf", bufs=24) as mpool,
            tc.tile_pool(name="aggp", bufs=8) as aggpool,
            tc.tile_pool(name="rst", bufs=4) as rpool,
            tc.tile_pool(name="pacc", bufs=3, space="PSUM") as papool,
            tc.tile_pool(name="prst", bufs=2, space="PSUM") as prpool,
        ):
            # ---- constant / setup loads ----
            w_sb = cpool.tile([128, T_total], dt.float32)
            par_sb = cpool.tile([128, T_total], dt.float32)
            ocnt_sb = cpool.tile([128, T_total], dt.int16)
            icnt_sb = cpool.tile([128, NB], dt.int16)
            gidx_sb = cpool.tile([128, T_total * 8], dt.int16)
            W_sb = cpool.tile([128, F], dt.float32)
            W_h = cpool.tile([128, F], dt.float16)
            b_sb = cpool.tile([1, F], dt.float32)
            ones1 = cpool.tile([1, F], dt.float32)
            b_bcast = cpool.tile([128, F], dt.float32)
            mlow = cpool.tile([128, T_total], dt.float32)
            mhigh = cpool.tile([128, T_total], dt.float32)
            rs_in = cpool.tile([128, NB], dt.float32)
            tmp_f = cpool.tile([128, T_total], dt.float32)
            tmp_g = cpool.tile([128, T_total], dt.float32)
            tmp_i = cpool.tile([128, NB], dt.float32)
            tmp_j = cpool.tile([128, NB], dt.float32)

            nc.sync.dma_start(out=w_sb[:], in_=wv_d[:])
            nc.sync.dma_start(out=par_sb[:], in_=par_d[:])
            nc.sync.dma_start(out=ocnt_sb[:], in_=ocnt_d[:])
            nc.sync.dma_start(out=icnt_sb[:], in_=icnt_d[:])
            nc.sync.dma_start(out=gidx_sb[:], in_=gidx_d[:])
            nc.sync.dma_start(out=W_sb[:], in_=W_d[:])
            nc.sync.dma_start(out=b_sb[:], in_=b_d[:])

            nc.scalar.activation(out=W_h[:], in_=W_sb[:], func=AF.Copy)

            # broadcast b across partitions via a K=1 outer-product matmul
            nc.vector.memset(ones1[:], 1.0)
            pb = prpool.tile([128, F], dt.float32, space="PSUM")
            nc.tensor.matmul(pb[:], ones1[:], b_sb[:], start=True, stop=True)
            nc.vector.tensor_copy(out=b_bcast[:], in_=pb[:])

            # s_e = w_e * rsqrt(outdeg_e); split by pair parity into mlow/mhigh
            nc.vector.tensor_copy(out=tmp_f[:], in_=ocnt_sb[:])
            nc.vector.reciprocal(out=tmp_g[:], in_=tmp_f[:])
            nc.scalar.activation(out=tmp_f[:], in_=tmp_g[:], func=AF.Sqrt)
            nc.vector.tensor_tensor(out=tmp_g[:], in0=w_sb[:], in1=tmp_f[:], op=ALU.mult)
            nc.vector.tensor_tensor(out=mhigh[:], in0=tmp_g[:], in1=par_sb[:], op=ALU.mult)
            nc.vector.tensor_tensor(out=mlow[:], in0=tmp_g[:], in1=mhigh[:], op=ALU.subtract)

            # rs_in = rsqrt(max(indeg, 1))
            nc.vector.tensor_copy(out=tmp_i[:], in_=icnt_sb[:])
            nc.vector.tensor_scalar_max(tmp_j[:], tmp_i[:], 1.0)
            nc.vector.reciprocal(out=tmp_i[:], in_=tmp_j[:])
            nc.scalar.activation(out=rs_in[:], in_=tmp_i[:], func=AF.Sqrt)

            # ---- main aggregation, finalization inlined per block ----
            pa = None
            for c0 in range(0, T_total, CHUNK_T):
                ct = min(CHUNK_T, T_total - c0)
                gbuf = gpool.tile([128, CHUNK_T * 256], dt.float32, tag="gbuf")
                selc = selpool.tile([128, CHUNK_T * 128], dt.float16, tag="selc")
                with tc.high_priority():
                    nc.gpsimd.dma_gather(
                        gbuf[:, :ct * 256].rearrange("p (t e) -> p t e", e=256),
                        feat_pairs,
                        gidx_sb[:, c0 * 8:(c0 + ct) * 8],
                        ct * 128,
                        ct * 128,
                        256,
                    )
                nc.sync.dma_start(
                    out=selc[:, :ct * 128], in_=sel_d[:, c0 * 128:(c0 + ct) * 128],
                )
                for t in range(ct):
                    gt = c0 + t
                    bk = int(tile_blk[gt])
                    ti = int(tile_in_blk[gt])
                    last = ti == T_b[bk] - 1
                    ml = mpool.tile([128, 128], dt.float16, tag="ml")
                    mh = mpool.tile([128, 128], dt.float16, tag="mh")
                    nc.scalar.activation(
                        out=ml[:], in_=gbuf[:, t * 256:t * 256 + 128],
                        func=AF.Copy, scale=mlow[:, gt:gt + 1],
                    )
                    nc.vector.tensor_scalar(
                        mh[:], gbuf[:, t * 256 + 128:t * 256 + 256],
                        mhigh[:, gt:gt + 1], None, op0=ALU.mult,
                    )
                    if ti == 0:
                        pa = papool.tile([128, 128], dt.float32, space="PSUM", tag="pa")
                    nc.tensor.matmul(pa[:], ml[:], selc[:, t * 128:(t + 1) * 128],
                                     start=(ti == 0), stop=False)
                    nc.tensor.matmul(pa[:], mh[:], selc[:, t * 128:(t + 1) * 128],
                                     start=False, stop=last)
                    if last:
                        # flush + finalize this block inline
                        aggTh = aggpool.tile([128, F], dt.float16, tag="aggTh")
                        nc.scalar.activation(out=aggTh[:], in_=pa[:], func=AF.Copy)
                        pr = prpool.tile([128, F], dt.float32, space="PSUM", tag="pr")
                        nc.tensor.matmul(pr[:], aggTh[:], W_h[:], start=True, stop=True)
                        rt = rpool.tile([128, F], dt.float32, tag="rt")
                        nc.scalar.activation(
                            out=rt[:], in_=pr[:], func=AF.Copy,
                            scale=rs_in[:, bk:bk + 1],
                        )
                        nc.vector.tensor_tensor(out=rt[:], in0=rt[:], in1=b_bcast[:],
                                                op=ALU.add)
                        n0 = bk * 128
                        n1 = min(n0 + 128, NPC)
                        nc.sync.dma_start(out=out_d[n0:n1, :], in_=rt[:n1 - n0, :])

    nc.compile()
    return nc


def kernel(feat, W, b, edge_w, edge_src, edge_dst):
    from concourse.bass_utils import run_bass_kernel_spmd

    T_b, off, in_maps = _host_prep(feat, W, b, edge_w, edge_src, edge_dst)

    key = tuple(T_b)
    if key not in _BUILD_CACHE:
        _BUILD_CACHE[key] = _build_program(T_b, off)
    nc = _BUILD_CACHE[key]

    trace = bool(int(os.environ.get("GCN_TRACE", "0")))
    res = run_bass_kernel_spmd(
        nc, in_maps, core_ids=list(range(C)),
        trace=trace,
        trace_cores=list(range(C)) if trace else None,
    )
    kernel.last_results = res
    out = np.concatenate([r["out"] for r in res.results], axis=0)
    return out.astype(np.float32)


# revision 10
# speedup vs baseline: 2.2072x; 1.8920x over previous
"""GCN layer (gather + scale + segment-sum + 128x128 matmul) on 8 TRN2 NeuronCores.

Sharding: nodes (and their incident edges, partitioned by dst) are sharded
across the 8 cores; the 128x128 weight is replicated. Per core (~100K edges):

  host (integer/permutation/structure preprocessing only):
    - select edges with dst in the core's 6250-row slice, sort by
      (src>=32768, dst_block) so each 128-edge tile maps to one 128-dst
      block and gather indices fit int16 (low/high base split)
    - pad each (phase, block) edge group to a multiple of 128 with
      (idx=0, w=0, empty selector row); per-(phase, block) tile counts are
      maxed across cores so all 8 cores run one SPMD program
    - the 0/1 one-hot selector tiles (edge -> dst_local routing, pure graph
      structure) are shipped as an fp16 input and streamed by regular DMA
    - per-edge out-degree counts / per-node in-degree counts are shipped as
      int16 (index bookkeeping); all float math happens on device

  device:
    - s_e = edge_w * rsqrt(outdeg[src_e])                          (DVE+ACT)
    - batched dma_gather of feat rows by src, 512B elems, round-robined
      over all 4 SWDGE queues so descriptor generation runs on all four
      GpSimd Q7 core pairs in parallel                             (SWDGE)
    - per tile: fp16 cast scaled by s_e, alternating ACT / DVE
    - aggT[f, d] += msg_tile^T @ sel_tile  accumulated in PSUM     (PE)
    - per block: rst = (aggT^T @ W) * rsqrt(max(indeg,1)) + b, finalized
      inline so it overlaps the gather stream  (PE + ACT + DVE)
"""

import os
import numpy as np

N_NODES = 50000
N_EDGES = 800000
F = 128
C = 8
NPC = N_NODES // C          # 6250 nodes per core
NB = (NPC + 127) // 128     # 49 dst blocks per core (48 full + 106)
SPLIT = 32768               # int16 gather-index base split
CHUNK_T = 8                 # tiles per dma_gather (1024-idx ucode packet limit)
N_QUEUES = 4                # SWDGE queues (parallel Q7 desc-gen core pairs)


def _host_prep(feat, W, b, edge_w, edge_src, edge_dst):
    src = np.ascontiguousarray(np.asarray(edge_src)).astype(np.int64)
    dst = np.ascontiguousarray(np.asarray(edge_dst)).astype(np.int64)
    w = np.ascontiguousarray(np.asarray(edge_w)).astype(np.float32)

    outcnt = np.bincount(src, minlength=N_NODES)

    per_core = []
    core_of = dst // NPC
    for c in range(C):
        m = core_of == c
        s_c = src[m]
        d_c = dst[m] - c * NPC
        w_c = w[m]
        blk = d_c >> 7
        hi = (s_c >= SPLIT).astype(np.int64)
        perm = np.lexsort((blk, hi))
        per_core.append((s_c[perm], d_c[perm], w_c[perm], blk[perm], hi[perm]))

    cnts = np.zeros((C, 2, NB), np.int64)
    for c in range(C):
        _, _, _, blk, hi = per_core[c]
        for p in range(2):
            cnts[c, p] = np.bincount(blk[hi == p], minlength=NB)
    T_pb = np.maximum(1, (cnts.max(axis=0) + 127) // 128)  # [2, NB]
    off = np.zeros((2, NB), np.int64)
    cur = 0
    for p in range(2):
        for bk in range(NB):
            off[p, bk] = cur
            cur += T_pb[p, bk]
    T_total = int(cur)

    in_maps = []
    for c in range(C):
        s_c, d_c, w_c, blk, hi = per_core[c]
        gidx = np.zeros(T_total * 128, np.int64)
        wv = np.zeros(T_total * 128, np.float32)
        dstl = np.full(T_total * 128, -1, np.int64)
        cnt = np.ones(T_total * 128, np.int64)
        order_key = hi * NB + blk
        grp_cnt = np.bincount(order_key, minlength=2 * NB)
        grp_start = np.concatenate([[0], np.cumsum(grp_cnt)])
        for p in range(2):
            for bk in range(NB):
                g = p * NB + bk
                e0, e1 = grp_start[g], grp_start[g + 1]
                k = e1 - e0
                s0 = off[p, bk] * 128
                gidx[s0:s0 + k] = s_c[e0:e1] - p * SPLIT
                wv[s0:s0 + k] = w_c[e0:e1]
                dstl[s0:s0 + k] = d_c[e0:e1] - bk * 128
                cnt[s0:s0 + k] = outcnt[s_c[e0:e1]]
        icnt = np.bincount(d_c, minlength=NPC)
        icnt_pad = np.ones(NB * 128, np.int64)
        icnt_pad[:NPC] = icnt

        idx_wrapped = np.tile(gidx.reshape(-1, 16).T.astype(np.int16), (8, 1))

        # one-hot selector tiles: sel[128e, T*128] fp16, laid out tile-major.
        dstl_t = dstl.reshape(T_total, 128)          # [T, e]
        sel_host = np.zeros((128, T_total * 128), np.float16)
        tt, ee = np.nonzero(dstl_t >= 0)
        sel_host[ee, tt * 128 + dstl_t[tt, ee]] = np.float16(1.0)

        in_maps.append({
            "feat": np.ascontiguousarray(np.asarray(feat, np.float32)),
            "Wm": np.ascontiguousarray(np.asarray(W, np.float32)),
            "bv": np.ascontiguousarray(np.asarray(b, np.float32).reshape(1, F)),
            "wv": np.ascontiguousarray(wv.reshape(T_total, 128).T),
            "ocnt": np.ascontiguousarray(cnt.reshape(T_total, 128).T.astype(np.int16)),
            "icnt": np.ascontiguousarray(icnt_pad.reshape(NB, 128).T.astype(np.int16)),
            "gidx": np.ascontiguousarray(idx_wrapped),
            "selh": np.ascontiguousarray(sel_host),
        })
    return T_pb, off, in_maps


_BUILD_CACHE = {}


def _build_program(T_pb, off):
    import concourse.bacc as bacc
    import concourse.mybir as mybir
    import concourse.tile as tile
    from concourse._compat import get_trn_type

    dt = mybir.dt
    AF = mybir.ActivationFunctionType
    ALU = mybir.AluOpType

    TLs = int(T_pb[0].sum())
    THs = int(T_pb[1].sum())
    T_total = TLs + THs

    nc = bacc.Bacc(get_trn_type() or "TRN2", target_bir_lowering=False, debug=False,
                   num_swdge_queues=N_QUEUES)

    feat_d = nc.dram_tensor("feat", [N_NODES, F], dt.float32, kind="ExternalInput")
    W_d = nc.dram_tensor("Wm", [F, F], dt.float32, kind="ExternalInput")
    b_d = nc.dram_tensor("bv", [1, F], dt.float32, kind="ExternalInput")
    wv_d = nc.dram_tensor("wv", [128, T_total], dt.float32, kind="ExternalInput")
    ocnt_d = nc.dram_tensor("ocnt", [128, T_total], dt.int16, kind="ExternalInput")
    icnt_d = nc.dram_tensor("icnt", [128, NB], dt.int16, kind="ExternalInput")
    gidx_d = nc.dram_tensor("gidx", [128, T_total * 8], dt.int16, kind="ExternalInput")
    sel_d = nc.dram_tensor("selh", [128, T_total * 128], dt.float16, kind="ExternalInput")
    out_d = nc.dram_tensor("out", [NPC, F], dt.float32, kind="ExternalOutput")

    # per-phase tile -> block mapping
    tile_blk = [np.repeat(np.arange(NB), T_pb[p]) for p in range(2)]
    tile_in_blk = [np.concatenate([np.arange(T_pb[p][bk]) for bk in range(NB)])
                   for p in range(2)]

    with tile.TileContext(nc) as tc:
        with (
            tc.tile_pool(name="const", bufs=1) as cpool,
            tc.tile_pool(name="gbuf", bufs=12) as gpool,
            tc.tile_pool(name="selbuf", bufs=8) as selpool,
            tc.tile_pool(name="mbuf", bufs=24) as mpool,
            tc.tile_pool(name="aggp", bufs=8) as aggpool,
            tc.tile_pool(name="rst", bufs=4) as rpool,
            tc.tile_pool(name="pacc", bufs=3, space="PSUM") as papool,
            tc.tile_pool(name="prst", bufs=2, space="PSUM") as prpool,
        ):
            # ---- constant / setup loads ----
            w_sb = cpool.tile([128, T_total], dt.float32)
            ocnt_sb = cpool.tile([128, T_total], dt.int16)
            icnt_sb = cpool.tile([128, NB], dt.int16)
            gidx_sb = cpool.tile([128, T_total * 8], dt.int16)
            W_sb = cpool.tile([128, F], dt.float32)
            W_h = cpool.tile([128, F], dt.float16)
            b_sb = cpool.tile([1, F], dt.float32)
            ones1 = cpool.tile([1, F], dt.float32)
            b_bcast = cpool.tile([128, F], dt.float32)
            s_sb = cpool.tile([128, T_total], dt.float32)
            aggL = cpool.tile([128, NB * 128], dt.float32)
            rs_in = cpool.tile([128, NB], dt.float32)
            tmp_f = cpool.tile([128, T_total], dt.float32)
            tmp_g = cpool.tile([128, T_total], dt.float32)
            tmp_i = cpool.tile([128, NB], dt.float32)
            tmp_j = cpool.tile([128, NB], dt.float32)

            nc.sync.dma_start(out=w_sb[:], in_=wv_d[:])
            nc.sync.dma_start(out=ocnt_sb[:], in_=ocnt_d[:])
            nc.sync.dma_start(out=icnt_sb[:], in_=icnt_d[:])
            nc.sync.dma_start(out=gidx_sb[:], in_=gidx_d[:])
            nc.sync.dma_start(out=W_sb[:], in_=W_d[:])
            nc.sync.dma_start(out=b_sb[:], in_=b_d[:])

            nc.scalar.activation(out=W_h[:], in_=W_sb[:], func=AF.Copy)

            # broadcast b across partitions via a K=1 outer-product matmul
            nc.vector.memset(ones1[:], 1.0)
            pb = prpool.tile([128, F], dt.float32, space="PSUM")
            nc.tensor.matmul(pb[:], ones1[:], b_sb[:], start=True, stop=True)
            nc.vector.tensor_copy(out=b_bcast[:], in_=pb[:])

            # s_e = w_e * rsqrt(outdeg_e)
            nc.vector.tensor_copy(out=tmp_f[:], in_=ocnt_sb[:])
            nc.vector.reciprocal(out=tmp_g[:], in_=tmp_f[:])
            nc.scalar.activation(out=tmp_f[:], in_=tmp_g[:], func=AF.Sqrt)
            nc.vector.tensor_tensor(out=s_sb[:], in0=w_sb[:], in1=tmp_f[:], op=ALU.mult)

            # rs_in = rsqrt(max(indeg, 1))
            nc.vector.tensor_copy(out=tmp_i[:], in_=icnt_sb[:])
            nc.vector.tensor_scalar_max(tmp_j[:], tmp_i[:], 1.0)
            nc.vector.reciprocal(out=tmp_i[:], in_=tmp_j[:])
            nc.scalar.activation(out=rs_in[:], in_=tmp_i[:], func=AF.Sqrt)

            # ---- main aggregation, finalization inlined per block ----
            chunk_no = 0
            pa = None
            for p in range(2):
                Tp = TLs if p == 0 else THs
                base = p * TLs
                src_ap = feat_d[:, :] if p == 0 else feat_d[SPLIT:, :]
                for c0 in range(0, Tp, CHUNK_T):
                    ct = min(CHUNK_T, Tp - c0)
                    gbuf = gpool.tile([128, CHUNK_T * 128], dt.float32, tag="gbuf")
                    selc = selpool.tile([128, CHUNK_T * 128], dt.float16, tag="selc")
                    with tc.high_priority():
                        nc.gpsimd.dma_gather(
                            gbuf[:, :ct * 128].rearrange("p (t e) -> p t e", e=128),
                            src_ap,
                            gidx_sb[:, (base + c0) * 8:(base + c0 + ct) * 8],
                            ct * 128,
                            ct * 128,
                            128,
                            queue_num=chunk_no % N_QUEUES,
                        )
                    chunk_no += 1
                    nc.sync.dma_start(
                        out=selc[:, :ct * 128],
                        in_=sel_d[:, (base + c0) * 128:(base + c0 + ct) * 128],
                    )
                    for t in range(ct):
                        lt = c0 + t           # tile index within phase
                        gt = base + lt        # global tile index
                        bk = int(tile_blk[p][lt])
                        ti = int(tile_in_blk[p][lt])
                        last = ti == T_pb[p][bk] - 1
                        m = mpool.tile([128, 128], dt.float16, tag="m")
                        if gt % 2 == 0:
                            nc.scalar.activation(
                                out=m[:], in_=gbuf[:, t * 128:(t + 1) * 128],
                                func=AF.Copy, scale=s_sb[:, gt:gt + 1],
                            )
                        else:
                            nc.vector.tensor_scalar(
                                m[:], gbuf[:, t * 128:(t + 1) * 128],
                                s_sb[:, gt:gt + 1], None, op0=ALU.mult,
                            )
                        if ti == 0:
                            pa = papool.tile([128, 128], dt.float32, space="PSUM", tag="pa")
                        nc.tensor.matmul(pa[:], m[:], selc[:, t * 128:(t + 1) * 128],
                                         start=(ti == 0), stop=last)
                        if last and p == 0:
                            # stash phase-L partial in the SBUF f32 stash
                            nc.scalar.activation(
                                out=aggL[:, bk * 128:(bk + 1) * 128], in_=pa[:],
                                func=AF.Copy)
                        elif last:
                            # phase H: combine with phase-L partial and finalize
                            aggTh = aggpool.tile([128, F], dt.float16, tag="aggTh")
                            nc.vector.tensor_tensor(
                                out=aggTh[:], in0=pa[:],
                                in1=aggL[:, bk * 128:(bk + 1) * 128],
                                op=ALU.add,
                            )
                            pr = prpool.tile([128, F], dt.float32, space="PSUM", tag="pr")
                            nc.tensor.matmul(pr[:], aggTh[:], W_h[:], start=True, stop=True)
                            rt = rpool.tile([128, F], dt.float32, tag="rt")
                            nc.scalar.activation(
                                out=rt[:], in_=pr[:], func=AF.Copy,
                                scale=rs_in[:, bk:bk + 1],
                            )
                            nc.vector.tensor_tensor(out=rt[:], in0=rt[:], in1=b_bcast[:],
                                                    op=ALU.add)
                            n0 = bk * 128
                            n1 = min(n0 + 128, NPC)
                            nc.sync.dma_start(out=out_d[n0:n1, :], in_=rt[:n1 - n0, :])

    nc.compile()
    return nc


def kernel(feat, W, b, edge_w, edge_src, edge_dst):
    from concourse.bass_utils import run_bass_kernel_spmd

    T_pb, off, in_maps = _host_prep(feat, W, b, edge_w, edge_src, edge_dst)

    key = (tuple(T_pb[0]), tuple(T_pb[1]))
    if key not in _BUILD_CACHE:
        _BUILD_CACHE[key] = _build_program(T_pb, off)
    nc = _BUILD_CACHE[key]

    trace = bool(int(os.environ.get("GCN_TRACE", "0")))
    res = run_bass_kernel_spmd(
        nc, in_maps, core_ids=list(range(C)),
        trace=trace,
        trace_cores=list(range(C)) if trace else None,
    )
    kernel.last_results = res
    out = np.concatenate([r["out"] for r in res.results], axis=0)
    return out.astype(np.float32)


# revision 11
# speedup vs baseline: 2.9501x; 1.3366x over previous
"""GCN layer (gather + scale + segment-sum + 128x128 matmul) on 8 TRN2 NeuronCores.

Sharding: nodes (and their incident edges, partitioned by dst) are sharded
across the 8 cores; the 128x128 weight is replicated. Per core (~100K edges):

  host (integer/permutation/structure preprocessing only):
    - select edges with dst in the core's 6250-row slice, sort by
      (src>=32768, dst_block) so each 128-edge tile maps to one 128-dst
      block and gather indices fit int16 (low/high base split)
    - pad each (phase, block) edge group to a multiple of 128 with
      (idx=0, w=0, empty selector row); per-(phase, block) tile counts are
      maxed across cores so all 8 cores run one SPMD program
    - the 0/1 one-hot selector tiles (edge -> dst_local routing, pure graph
      structure) are shipped as an fp16 input and streamed by regular DMA
    - per-edge out-degree counts / per-node in-degree counts are shipped as
      int16 (index bookkeeping); all float math happens on device

  device:
    - s_e = edge_w * rsqrt(outdeg[src_e])                          (DVE+ACT)
    - batched dma_gather of feat rows by src, 512B elems, round-robined
      over all 4 SWDGE queues so descriptor generation runs on all four
      GpSimd Q7 core pairs in parallel                             (SWDGE)
    - per tile: fp16 cast scaled by s_e, alternating ACT / DVE
    - aggT[f, d] += msg_tile^T @ sel_tile  accumulated in PSUM     (PE)
    - per block: rst = (aggT^T @ W) * rsqrt(max(indeg,1)) + b, finalized
      inline so it overlaps the gather stream  (PE + ACT + DVE)
"""

import os
import numpy as np

N_NODES = 50000
N_EDGES = 800000
F = 128
C = 8
NPC = N_NODES // C          # 6250 nodes per core
NB = (NPC + 127) // 128     # 49 dst blocks per core (48 full + 106)
SPLIT = 32768               # int16 gather-index base split
CHUNK_T = 8                 # tiles per dma_gather (1024-idx ucode packet limit)
N_QUEUES = 4                # SWDGE queues (parallel Q7 desc-gen core pairs)


def _host_prep(feat, W, b, edge_w, edge_src, edge_dst):
    src = np.ascontiguousarray(np.asarray(edge_src)).astype(np.int64)
    dst = np.ascontiguousarray(np.asarray(edge_dst)).astype(np.int64)
    w = np.ascontiguousarray(np.asarray(edge_w)).astype(np.float32)

    outcnt = np.bincount(src, minlength=N_NODES)

    per_core = []
    core_of = dst // NPC
    for c in range(C):
        m = core_of == c
        s_c = src[m]
        d_c = dst[m] - c * NPC
        w_c = w[m]
        blk = d_c >> 7
        hi = (s_c >= SPLIT).astype(np.int64)
        perm = np.lexsort((blk, hi))
        per_core.append((s_c[perm], d_c[perm], w_c[perm], blk[perm], hi[perm]))

    cnts = np.zeros((C, 2, NB), np.int64)
    for c in range(C):
        _, _, _, blk, hi = per_core[c]
        for p in range(2):
            cnts[c, p] = np.bincount(blk[hi == p], minlength=NB)
    T_pb = np.maximum(1, (cnts.max(axis=0) + 127) // 128)  # [2, NB]
    off = np.zeros((2, NB), np.int64)
    cur = 0
    for p in range(2):
        for bk in range(NB):
            off[p, bk] = cur
            cur += T_pb[p, bk]
    T_total = int(cur)

    in_maps = []
    for c in range(C):
        s_c, d_c, w_c, blk, hi = per_core[c]
        gidx = np.zeros(T_total * 128, np.int64)
        wv = np.zeros(T_total * 128, np.float32)
        dstl = np.full(T_total * 128, -1, np.int64)
        cnt = np.ones(T_total * 128, np.int64)
        order_key = hi * NB + blk
        grp_cnt = np.bincount(order_key, minlength=2 * NB)
        grp_start = np.concatenate([[0], np.cumsum(grp_cnt)])
        for p in range(2):
            for bk in range(NB):
                g = p * NB + bk
                e0, e1 = grp_start[g], grp_start[g + 1]
                k = e1 - e0
                s0 = off[p, bk] * 128
                gidx[s0:s0 + k] = s_c[e0:e1] - p * SPLIT
                wv[s0:s0 + k] = w_c[e0:e1]
                dstl[s0:s0 + k] = d_c[e0:e1] - bk * 128
                cnt[s0:s0 + k] = outcnt[s_c[e0:e1]]
        icnt = np.bincount(d_c, minlength=NPC)
        icnt_pad = np.ones(NB * 128, np.int64)
        icnt_pad[:NPC] = icnt

        idx_wrapped = np.tile(gidx.reshape(-1, 16).T.astype(np.int16), (8, 1))

        # one-hot selector tiles: sel[128e, T*128] fp16, laid out tile-major.
        dstl_t = dstl.reshape(T_total, 128)          # [T, e]
        sel_host = np.zeros((128, T_total * 128), np.float16)
        tt, ee = np.nonzero(dstl_t >= 0)
        sel_host[ee, tt * 128 + dstl_t[tt, ee]] = np.float16(1.0)

        in_maps.append({
            "feat": np.ascontiguousarray(np.asarray(feat, np.float32)),
            "Wm": np.ascontiguousarray(np.asarray(W, np.float32)),
            "bv": np.ascontiguousarray(np.asarray(b, np.float32).reshape(1, F)),
            "wv": np.ascontiguousarray(wv.reshape(T_total, 128).T),
            "ocnt": np.ascontiguousarray(cnt.reshape(T_total, 128).T.astype(np.int16)),
            "icnt": np.ascontiguousarray(icnt_pad.reshape(NB, 128).T.astype(np.int16)),
            "gidx": np.ascontiguousarray(idx_wrapped),
            "selh": np.ascontiguousarray(sel_host),
        })
    return T_pb, off, in_maps


_BUILD_CACHE = {}


def _build_program(T_pb, off):
    import concourse.bacc as bacc
    import concourse.mybir as mybir
    import concourse.tile as tile
    from concourse._compat import get_trn_type

    dt = mybir.dt
    AF = mybir.ActivationFunctionType
    ALU = mybir.AluOpType

    TLs = int(T_pb[0].sum())
    THs = int(T_pb[1].sum())
    T_total = TLs + THs

    nc = bacc.Bacc(get_trn_type() or "TRN2", target_bir_lowering=False, debug=False,
                   num_swdge_queues=N_QUEUES)

    feat_d = nc.dram_tensor("feat", [N_NODES, F], dt.float32, kind="ExternalInput")
    W_d = nc.dram_tensor("Wm", [F, F], dt.float32, kind="ExternalInput")
    b_d = nc.dram_tensor("bv", [1, F], dt.float32, kind="ExternalInput")
    wv_d = nc.dram_tensor("wv", [128, T_total], dt.float32, kind="ExternalInput")
    ocnt_d = nc.dram_tensor("ocnt", [128, T_total], dt.int16, kind="ExternalInput")
    icnt_d = nc.dram_tensor("icnt", [128, NB], dt.int16, kind="ExternalInput")
    gidx_d = nc.dram_tensor("gidx", [128, T_total * 8], dt.int16, kind="ExternalInput")
    sel_d = nc.dram_tensor("selh", [128, T_total * 128], dt.float16, kind="ExternalInput")
    out_d = nc.dram_tensor("out", [NPC, F], dt.float32, kind="ExternalOutput")

    # per-phase tile -> block mapping
    tile_blk = [np.repeat(np.arange(NB), T_pb[p]) for p in range(2)]
    tile_in_blk = [np.concatenate([np.arange(T_pb[p][bk]) for bk in range(NB)])
                   for p in range(2)]

    with tile.TileContext(nc) as tc:
        with (
            tc.tile_pool(name="const", bufs=1) as cpool,
            tc.tile_pool(name="gbuf", bufs=12) as gpool,
            tc.tile_pool(name="selbuf", bufs=8) as selpool,
            tc.tile_pool(name="mbuf", bufs=24) as mpool,
            tc.tile_pool(name="aggp", bufs=8) as aggpool,
            tc.tile_pool(name="rst", bufs=4) as rpool,
            tc.tile_pool(name="pacc", bufs=3, space="PSUM") as papool,
            tc.tile_pool(name="prst", bufs=2, space="PSUM") as prpool,
        ):
            # ---- constant / setup loads ----
            w_sb = cpool.tile([128, T_total], dt.float32)
            ocnt_sb = cpool.tile([128, T_total], dt.int16)
            icnt_sb = cpool.tile([128, NB], dt.int16)
            gidx_sb = cpool.tile([128, T_total * 8], dt.int16)
            W_sb = cpool.tile([128, F], dt.float32)
            W_h = cpool.tile([128, F], dt.float16)
            b_sb = cpool.tile([1, F], dt.float32)
            ones1 = cpool.tile([1, F], dt.float32)
            b_bcast = cpool.tile([128, F], dt.float32)
            s_sb = cpool.tile([128, T_total], dt.float32)
            aggL = cpool.tile([128, NB * 128], dt.float32)
            rs_in = cpool.tile([128, NB], dt.float32)
            tmp_f = cpool.tile([128, T_total], dt.float32)
            tmp_g = cpool.tile([128, T_total], dt.float32)
            tmp_i = cpool.tile([128, NB], dt.float32)
            tmp_j = cpool.tile([128, NB], dt.float32)

            nc.sync.dma_start(out=w_sb[:], in_=wv_d[:])
            nc.sync.dma_start(out=ocnt_sb[:], in_=ocnt_d[:])
            nc.sync.dma_start(out=icnt_sb[:], in_=icnt_d[:])
            nc.sync.dma_start(out=gidx_sb[:], in_=gidx_d[:])
            nc.sync.dma_start(out=W_sb[:], in_=W_d[:])
            nc.sync.dma_start(out=b_sb[:], in_=b_d[:])

            nc.scalar.activation(out=W_h[:], in_=W_sb[:], func=AF.Copy)

            # broadcast b across partitions via a K=1 outer-product matmul
            nc.vector.memset(ones1[:], 1.0)
            pb = prpool.tile([128, F], dt.float32, space="PSUM")
            nc.tensor.matmul(pb[:], ones1[:], b_sb[:], start=True, stop=True)
            nc.vector.tensor_copy(out=b_bcast[:], in_=pb[:])

            # s_e = w_e * rsqrt(outdeg_e)
            nc.vector.tensor_copy(out=tmp_f[:], in_=ocnt_sb[:])
            nc.vector.reciprocal(out=tmp_g[:], in_=tmp_f[:])
            nc.scalar.activation(out=tmp_f[:], in_=tmp_g[:], func=AF.Sqrt)
            nc.vector.tensor_tensor(out=s_sb[:], in0=w_sb[:], in1=tmp_f[:], op=ALU.mult)

            # rs_in = rsqrt(max(indeg, 1))
            nc.vector.tensor_copy(out=tmp_i[:], in_=icnt_sb[:])
            nc.vector.tensor_scalar_max(tmp_j[:], tmp_i[:], 1.0)
            nc.vector.reciprocal(out=tmp_i[:], in_=tmp_j[:])
            nc.scalar.activation(out=rs_in[:], in_=tmp_i[:], func=AF.Sqrt)

            # ---- main aggregation, finalization inlined per block ----
            chunk_no = 0
            pa = None
            for p in range(2):
                Tp = TLs if p == 0 else THs
                base = p * TLs
                src_ap = feat_d[:, :] if p == 0 else feat_d[SPLIT:, :]
                for c0 in range(0, Tp, CHUNK_T):
                    ct = min(CHUNK_T, Tp - c0)
                    gbuf = gpool.tile([128, CHUNK_T * 128], dt.float32, tag="gbuf")
                    selc = selpool.tile([128, CHUNK_T * 128], dt.float16, tag="selc")
                    with tc.high_priority():
                        nc.gpsimd.dma_gather(
                            gbuf[:, :ct * 128].rearrange("p (t e) -> p t e", e=128),
                            src_ap,
                            gidx_sb[:, (base + c0) * 8:(base + c0 + ct) * 8],
                            ct * 128,
                            ct * 128,
                            128,
                            queue_num=chunk_no % N_QUEUES,
                        )
                    chunk_no += 1
                    nc.sync.dma_start(
                        out=selc[:, :ct * 128],
                        in_=sel_d[:, (base + c0) * 128:(base + c0 + ct) * 128],
                    )
                    for t in range(ct):
                        lt = c0 + t           # tile index within phase
                        gt = base + lt        # global tile index
                        bk = int(tile_blk[p][lt])
                        ti = int(tile_in_blk[p][lt])
                        last = ti == T_pb[p][bk] - 1
                        m = mpool.tile([128, 128], dt.float16, tag="m")
                        if gt % 2 == 0:
                            nc.scalar.activation(
                                out=m[:], in_=gbuf[:, t * 128:(t + 1) * 128],
                                func=AF.Copy, scale=s_sb[:, gt:gt + 1],
                            )
                        else:
                            nc.vector.tensor_tensor(
                                out=m[:], in0=gbuf[:, t * 128:(t + 1) * 128],
                                in1=s_sb[:, gt:gt + 1].to_broadcast([128, 128]),
                                op=ALU.mult,
                            )
                        if ti == 0:
                            pa = papool.tile([128, 128], dt.float32, space="PSUM", tag="pa")
                        nc.tensor.matmul(pa[:], m[:], selc[:, t * 128:(t + 1) * 128],
                                         start=(ti == 0), stop=last)
                        if last and p == 0:
                            # stash phase-L partial in the SBUF f32 stash
                            nc.scalar.activation(
                                out=aggL[:, bk * 128:(bk + 1) * 128], in_=pa[:],
                                func=AF.Copy)
                        elif last:
                            # phase H: combine with phase-L partial and finalize
                            aggTh = aggpool.tile([128, F], dt.float16, tag="aggTh")
                            nc.vector.tensor_tensor(
                                out=aggTh[:], in0=pa[:],
                                in1=aggL[:, bk * 128:(bk + 1) * 128],
                                op=ALU.add,
                            )
                            pr = prpool.tile([128, F], dt.float32, space="PSUM", tag="pr")
                            nc.tensor.matmul(pr[:], aggTh[:], W_h[:], start=True, stop=True)
                            rt = rpool.tile([128, F], dt.float32, tag="rt")
                            nc.scalar.activation(
                                out=rt[:], in_=pr[:], func=AF.Copy,
                                scale=rs_in[:, bk:bk + 1],
                            )
                            nc.vector.tensor_tensor(out=rt[:], in0=rt[:], in1=b_bcast[:],
                                                    op=ALU.add)
                            n0 = bk * 128
                            n1 = min(n0 + 128, NPC)
                            nc.sync.dma_start(out=out_d[n0:n1, :], in_=rt[:n1 - n0, :])

    nc.compile()
    return nc


def kernel(feat, W, b, edge_w, edge_src, edge_dst):
    from concourse.bass_utils import run_bass_kernel_spmd

    T_pb, off, in_maps = _host_prep(feat, W, b, edge_w, edge_src, edge_dst)

    key = (tuple(T_pb[0]), tuple(T_pb[1]))
    if key not in _BUILD_CACHE:
        _BUILD_CACHE[key] = _build_program(T_pb, off)
    nc = _BUILD_CACHE[key]

    trace = bool(int(os.environ.get("GCN_TRACE", "0")))
    res = run_bass_kernel_spmd(
        nc, in_maps, core_ids=list(range(C)),
        trace=trace,
        trace_cores=list(range(C)) if trace else None,
    )
    kernel.last_results = res
    out = np.concatenate([r["out"] for r in res.results], axis=0)
    return out.astype(np.float32)
